# revision 16
# baseline (speedup 1.0000x reference)
"""Center-loss kernel for Trainium2 (Bass/Bacc, raw), 8-core data-parallel.

loss = 2 - 2 * (sum_i feature[i, label[i]] / 64) / 8192

Sharding: batch dim (8192 rows) split evenly across 8 NeuronCores.
Each core gathers its 1024 picked elements straight out of DRAM with
indirect DMAs (no full-matrix read), reduces them to one scalar partial
sum on-device, and the host combines the 8 partials.

Implementation notes (hard-won on this toolchain):
- Raw Bacc instead of TileContext: Tile's sem-init preamble + tail
  barrier butterfly cost ~16us on a ~4us kernel. Everything here runs
  on the GPSIMD (Pool) engine in-order with ONE DMA semaphore.
- The HW indirect DMA consumes ONE index per dest partition and fetches
  dest-free-size contiguous elements, so gathering 1024 scattered
  elements takes 8 DMAs of [128,1] (index tile column by column).
- labpack packs labels + constant row offsets so the index add is a
  single tensor_tensor with one sem wait (HW: 1 sync wait per inst).
"""

import sys

if "/opt/trn_rl_repo" not in sys.path:
    sys.path.insert(0, "/opt/trn_rl_repo")

import numpy as np

import concourse.bacc as bacc
import concourse.bass as bass
from concourse import mybir
from concourse import bass_utils

N = 8192          # batch rows
C = 10000         # num classes (feature columns)
N_CORES = 8
R = N // N_CORES  # rows per core
P = 128           # SBUF partitions
K = R // P        # picked elements per partition
SCALE = 64.0

_NC_CACHE = None


def _build_nc() -> bacc.Bacc:
    global _NC_CACHE
    if _NC_CACHE is not None:
        return _NC_CACHE

    nc = bacc.Bacc(trn_type="TRN2")
    feat = nc.dram_tensor("feature", [R, C], mybir.dt.float32, kind="ExternalInput")
    # labpack[0] = labels, labpack[1] = arange(R)*C (constant row offsets).
    lab = nc.dram_tensor("labpack", [2, R], mybir.dt.int32, kind="ExternalInput")
    out = nc.dram_tensor("out", [1, 1], mybir.dt.float32, kind="ExternalOutput")

    with (
        nc.sbuf_tensor("lp", [P, 2, K], mybir.dt.int32) as lp,
        nc.sbuf_tensor("idx", [P, K], mybir.dt.int32) as idx,
        nc.sbuf_tensor("gat", [P, K], mybir.dt.float32) as gat,
        nc.sbuf_tensor("res", [1, 1], mybir.dt.float32) as res,
        nc.semaphore() as S,
        nc.Block() as block,
    ):

        @block.gpsimd
        def _(g):
            # Sems are not zeroed at alloc; make every run self-correcting.
            g.sem_clear(S)
            # Local row r = p*K + j lives at tile position [p, :, j].
            g.dma_start(
                out=lp[:], in_=lab[:].rearrange("t (p k) -> p t k", p=P)
            ).then_inc(S, 16)
            g.wait_ge(S, 16)
            # Flat element index of feature[r, label[r]] = r*C + label[r].
            g.tensor_tensor(
                out=idx[:],
                in0=lp[:, 0, :],
                in1=lp[:, 1, :],
                op=mybir.AluOpType.add,
            )
            # One index per partition per DMA: column j gathers rows p*K+j.
            for j in range(K):
                g.indirect_dma_start(
                    out=gat[:, j : j + 1],
                    out_offset=None,
                    in_=feat[:],
                    # axis=1 -> coef = prod(shape[2:]) = 1: indices are flat
                    # element offsets into the contiguous [R, C] block.
                    in_offset=bass.IndirectOffsetOnAxis(
                        ap=idx[:, j : j + 1], axis=1
                    ),
                ).then_inc(S, 16)
            g.wait_ge(S, 16 * (1 + K))
            # GPSIMD reduce over partitions + free dims -> scalar.
            g.tensor_reduce(
                out=res[:],
                in_=gat[:],
                axis=mybir.AxisListType.XYZWC,
                op=mybir.AluOpType.add,
            )
            g.dma_start(out=out[:], in_=res[:]).then_inc(S, 16)
            # Quiesce: every DMA this kernel issued has completed.
            g.wait_ge(S, 16 * (2 + K))

    nc.finalize()
    _NC_CACHE = nc
    return nc


def _run(feature: np.ndarray, label: np.ndarray, **spmd_kwargs):
    nc = _build_nc()
    feature = np.ascontiguousarray(feature, dtype=np.float32)
    lab32 = np.ascontiguousarray(np.asarray(label).astype(np.int32))
    assert feature.shape == (N, C), feature.shape
    assert lab32.shape == (N,), lab32.shape

    row_off = (np.arange(R, dtype=np.int32) * C).astype(np.int32)
    in_maps = [
        {
            "feature": feature[c * R : (c + 1) * R],
            "labpack": np.stack([lab32[c * R : (c + 1) * R], row_off]),
        }
        for c in range(N_CORES)
    ]
    res = bass_utils.run_bass_kernel_spmd(
        nc, in_maps, core_ids=list(range(N_CORES)), **spmd_kwargs
    )
    partials = np.array(
        [m["out"].reshape(()) for m in res.results], dtype=np.float32
    )
    total = np.float32(partials.sum(dtype=np.float32))
    loss = np.float32(2.0) - np.float32(2.0) * (total / np.float32(SCALE)) / np.float32(N)
    return np.asarray(loss, dtype=np.float32), res


def kernel(feature: np.ndarray, label: np.ndarray) -> np.ndarray:
    loss, _ = _run(feature, label)
    return loss


# revision 17
# speedup vs baseline: 1.0536x; 1.0536x over previous
"""Center-loss kernel for Trainium2 (Bass/Bacc, raw), 8-core data-parallel.

loss = 2 - 2 * (sum_i feature[i, label[i]] / 64) / 8192

Sharding: batch dim (8192 rows) split evenly across 8 NeuronCores.
Each core gathers its 1024 picked elements straight out of DRAM with
indirect DMAs (no full-matrix read), reduces them to one scalar partial
sum on-device, and the host combines the 8 partials.

Implementation notes (hard-won on this toolchain):
- Raw Bacc instead of TileContext: Tile's sem-init preamble + tail
  barrier butterfly cost ~16us on a ~4us kernel. Everything here runs
  on the GPSIMD (Pool) engine in-order with ONE DMA semaphore.
- The HW indirect DMA consumes ONE index per dest partition and fetches
  dest-free-size contiguous elements, so gathering 1024 scattered
  elements takes 8 DMAs of [128,1] (index tile column by column).
- labpack packs labels + constant row offsets so the index add is a
  single tensor_tensor with one sem wait (HW: 1 sync wait per inst).
"""

import sys

if "/opt/trn_rl_repo" not in sys.path:
    sys.path.insert(0, "/opt/trn_rl_repo")

import numpy as np

import concourse.bacc as bacc
import concourse.bass as bass
from concourse import mybir
from concourse import bass_utils

N = 8192          # batch rows
C = 10000         # num classes (feature columns)
N_CORES = 8
R = N // N_CORES  # rows per core
P = 128           # SBUF partitions
K = R // P        # picked elements per partition
SCALE = 64.0

_NC_CACHE = None


def _build_nc() -> bacc.Bacc:
    global _NC_CACHE
    if _NC_CACHE is not None:
        return _NC_CACHE

    nc = bacc.Bacc(trn_type="TRN2")
    feat = nc.dram_tensor("feature", [R, C], mybir.dt.float32, kind="ExternalInput")
    # labpack[0] = labels, labpack[1] = arange(R)*C (constant row offsets).
    lab = nc.dram_tensor("labpack", [2, R], mybir.dt.int32, kind="ExternalInput")
    out = nc.dram_tensor("out", [1, 1], mybir.dt.float32, kind="ExternalOutput")

    with (
        nc.sbuf_tensor("lp", [P, 2, K], mybir.dt.int32) as lp,
        nc.sbuf_tensor("idx", [P, K], mybir.dt.int32) as idx,
        nc.sbuf_tensor("gat", [1, R], mybir.dt.float32) as gat,
        nc.sbuf_tensor("res", [1, 1], mybir.dt.float32) as res,
        nc.semaphore() as S,
        nc.Block() as block,
    ):

        @block.gpsimd
        def _(g):
            # Sems are not zeroed at alloc; make every run self-correcting.
            g.sem_clear(S)
            # Local row r = p*K + j lives at tile position [p, :, j].
            g.dma_start(
                out=lp[:], in_=lab[:].rearrange("t (p k) -> p t k", p=P)
            ).then_inc(S, 16)
            g.wait_ge(S, 16)
            # Flat element index of feature[r, label[r]] = r*C + label[r].
            g.tensor_tensor(
                out=idx[:],
                in0=lp[:, 0, :],
                in1=lp[:, 1, :],
                op=mybir.AluOpType.add,
            )
            # Single indirect DMA for all 1024 picks. Walrus generates one
            # descriptor per entry of the dest AP's second-to-last dim, so a
            # [1, 1024, 1] dest on one partition yields 1024 single-element
            # descriptors; the offset tile is consumed partition-fastest
            # (a permutation of our [p, j] order — irrelevant for the sum).
            g.indirect_dma_start(
                out=gat[:].rearrange("p (n one) -> p n one", one=1),
                out_offset=None,
                in_=feat[:],
                # axis=1 -> coef = prod(shape[2:]) = 1: indices are flat
                # element offsets into the contiguous [R, C] block.
                in_offset=bass.IndirectOffsetOnAxis(ap=idx[:], axis=1),
            ).then_inc(S, 16)
            g.wait_ge(S, 32)
            # GPSIMD reduce over the single-partition row -> scalar.
            g.tensor_reduce(
                out=res[:],
                in_=gat[:],
                axis=mybir.AxisListType.XYZWC,
                op=mybir.AluOpType.add,
            )
            g.dma_start(out=out[:], in_=res[:]).then_inc(S, 16)
            # Quiesce: every DMA this kernel issued has completed.
            g.wait_ge(S, 48)

    nc.finalize()
    _NC_CACHE = nc
    return nc


def _run(feature: np.ndarray, label: np.ndarray, **spmd_kwargs):
    nc = _build_nc()
    feature = np.ascontiguousarray(feature, dtype=np.float32)
    lab32 = np.ascontiguousarray(np.asarray(label).astype(np.int32))
    assert feature.shape == (N, C), feature.shape
    assert lab32.shape == (N,), lab32.shape

    row_off = (np.arange(R, dtype=np.int32) * C).astype(np.int32)
    in_maps = [
        {
            "feature": feature[c * R : (c + 1) * R],
            "labpack": np.stack([lab32[c * R : (c + 1) * R], row_off]),
        }
        for c in range(N_CORES)
    ]
    res = bass_utils.run_bass_kernel_spmd(
        nc, in_maps, core_ids=list(range(N_CORES)), **spmd_kwargs
    )
    partials = np.array(
        [m["out"].reshape(()) for m in res.results], dtype=np.float32
    )
    total = np.float32(partials.sum(dtype=np.float32))
    loss = np.float32(2.0) - np.float32(2.0) * (total / np.float32(SCALE)) / np.float32(N)
    return np.asarray(loss, dtype=np.float32), res


def kernel(feature: np.ndarray, label: np.ndarray) -> np.ndarray:
    loss, _ = _run(feature, label)
    return loss


# revision 18
# speedup vs baseline: 1.2141x; 1.1523x over previous
"""Center-loss kernel for Trainium2 (Bass/Bacc, raw), 8-core data-parallel.

loss = 2 - 2 * (sum_i feature[i, label[i]] / 64) / 8192

Sharding: batch dim (8192 rows) split evenly across 8 NeuronCores.
Each core gathers its 1024 picked elements straight out of DRAM with
indirect DMAs (no full-matrix read), reduces them to one scalar partial
sum on-device, and the host combines the 8 partials.

Implementation notes (hard-won on this toolchain):
- Raw Bacc instead of TileContext: Tile's sem-init preamble + tail
  barrier butterfly cost ~16us on a ~4us kernel. Everything here runs
  on the GPSIMD (Pool) engine in-order with ONE DMA semaphore.
- The HW indirect DMA consumes ONE index per dest partition and fetches
  dest-free-size contiguous elements, so gathering 1024 scattered
  elements takes 8 DMAs of [128,1] (index tile column by column).
- labpack packs labels + constant row offsets so the index add is a
  single tensor_tensor with one sem wait (HW: 1 sync wait per inst).
"""

import sys

if "/opt/trn_rl_repo" not in sys.path:
    sys.path.insert(0, "/opt/trn_rl_repo")

import numpy as np

import concourse.bacc as bacc
import concourse.bass as bass
from concourse import mybir
from concourse import bass_utils

N = 8192          # batch rows
C = 10000         # num classes (feature columns)
N_CORES = 8
R = N // N_CORES  # rows per core
P = 128           # SBUF partitions
K = R // P        # picked elements per partition
SCALE = 64.0

_NC_CACHE = None


def _build_nc() -> bacc.Bacc:
    global _NC_CACHE
    if _NC_CACHE is not None:
        return _NC_CACHE

    nc = bacc.Bacc(trn_type="TRN2", monotonic_sem_count=0)
    feat = nc.dram_tensor("feature", [R, C], mybir.dt.float32, kind="ExternalInput")
    # labpack[0] = labels, labpack[1] = arange(R)*C (constant row offsets).
    lab = nc.dram_tensor("labpack", [2, R], mybir.dt.int32, kind="ExternalInput")
    out = nc.dram_tensor("out", [1, 1], mybir.dt.float32, kind="ExternalOutput")

    with (
        nc.sbuf_tensor("lp", [P, 2, K], mybir.dt.int32) as lp,
        nc.sbuf_tensor("idx", [P, K], mybir.dt.int32) as idx,
        nc.sbuf_tensor("gat", [1, R], mybir.dt.float32) as gat,
        nc.sbuf_tensor("res", [1, 1], mybir.dt.float32) as res,
        nc.semaphore() as S,
        nc.Block(no_gpsimd_drain=True) as block,
    ):
        # S timeline: 16 labpack DMA done; 32 gather done; 33 reduce done;
        # 49 out DMA done. SP clears S at the end so re-executions of the
        # loaded NEFF start from zero (NRT only zeroes sems at load).

        @block.sync
        def _(sp):
            # Local row r = p*K + j lives at tile position [p, :, j].
            sp.dma_start(
                out=lp[:], in_=lab[:].rearrange("t (p k) -> p t k", p=P)
            ).then_inc(S, 16)
            sp.wait_ge(S, 49)
            sp.sem_clear(S)

        @block.gpsimd
        def _(g):
            g.wait_ge(S, 16)
            # Flat element index of feature[r, label[r]] = r*C + label[r].
            g.tensor_tensor(
                out=idx[:],
                in0=lp[:, 0, :],
                in1=lp[:, 1, :],
                op=mybir.AluOpType.add,
            )
            # Single indirect DMA for all 1024 picks. Walrus generates one
            # descriptor per entry of the dest AP's second-to-last dim, so a
            # [1, 1024, 1] dest on one partition yields 1024 single-element
            # descriptors; the offset tile is consumed partition-fastest
            # (a permutation of our [p, j] order — irrelevant for the sum).
            g.indirect_dma_start(
                out=gat[:].rearrange("p (n one) -> p n one", one=1),
                out_offset=None,
                in_=feat[:],
                # axis=1 -> coef = prod(shape[2:]) = 1: indices are flat
                # element offsets into the contiguous [R, C] block.
                in_offset=bass.IndirectOffsetOnAxis(ap=idx[:], axis=1),
            ).then_inc(S, 16)

        @block.vector
        def _(v):
            v.wait_ge(S, 32)
            v.tensor_reduce(
                out=res[:],
                in_=gat[:],
                axis=mybir.AxisListType.X,
                op=mybir.AluOpType.add,
            ).then_inc(S, 1)

        @block.scalar
        def _(a):
            a.wait_ge(S, 33)
            a.dma_start(out=out[:], in_=res[:]).then_inc(S, 16)

    nc.finalize()
    _NC_CACHE = nc
    return nc


def _run(feature: np.ndarray, label: np.ndarray, **spmd_kwargs):
    nc = _build_nc()
    feature = np.ascontiguousarray(feature, dtype=np.float32)
    lab32 = np.ascontiguousarray(np.asarray(label).astype(np.int32))
    assert feature.shape == (N, C), feature.shape
    assert lab32.shape == (N,), lab32.shape

    row_off = (np.arange(R, dtype=np.int32) * C).astype(np.int32)
    in_maps = [
        {
            "feature": feature[c * R : (c + 1) * R],
            "labpack": np.stack([lab32[c * R : (c + 1) * R], row_off]),
        }
        for c in range(N_CORES)
    ]
    res = bass_utils.run_bass_kernel_spmd(
        nc, in_maps, core_ids=list(range(N_CORES)), **spmd_kwargs
    )
    partials = np.array(
        [m["out"].reshape(()) for m in res.results], dtype=np.float32
    )
    total = np.float32(partials.sum(dtype=np.float32))
    loss = np.float32(2.0) - np.float32(2.0) * (total / np.float32(SCALE)) / np.float32(N)
    return np.asarray(loss, dtype=np.float32), res


def kernel(feature: np.ndarray, label: np.ndarray) -> np.ndarray:
    loss, _ = _run(feature, label)
    return loss


# revision 23
# speedup vs baseline: 1.3210x; 1.0881x over previous
"""Center-loss kernel for Trainium2 (Bass/Bacc, raw), 8-core data-parallel.

loss = 2 - 2 * (sum_i feature[i, label[i]] / 64) / 8192

Sharding: batch dim (8192 rows) split evenly across 8 NeuronCores.
Each core gathers its 1024 picked elements straight out of DRAM with
indirect DMAs (no full-matrix read), reduces them to one scalar partial
sum on-device, and the host combines the 8 partials.

Implementation notes (hard-won on this toolchain):
- Raw Bacc instead of TileContext: Tile's sem-init preamble + tail
  barrier butterfly cost ~16us on a ~4us kernel. Everything here runs
  on the GPSIMD (Pool) engine in-order with ONE DMA semaphore.
- The HW indirect DMA consumes ONE index per dest partition and fetches
  dest-free-size contiguous elements, so gathering 1024 scattered
  elements takes 8 DMAs of [128,1] (index tile column by column).
- labpack packs labels + constant row offsets so the index add is a
  single tensor_tensor with one sem wait (HW: 1 sync wait per inst).
"""

import sys

if "/opt/trn_rl_repo" not in sys.path:
    sys.path.insert(0, "/opt/trn_rl_repo")

import numpy as np

import concourse.bacc as bacc
import concourse.bass as bass
from concourse import mybir
from concourse import bass_utils

N = 8192          # batch rows
C = 10000         # num classes (feature columns)
N_CORES = 8
R = N // N_CORES  # rows per core
P = 128           # SBUF partitions
K = R // P        # picked elements per partition
SCALE = 64.0

_NC_CACHE = None


def _build_nc() -> bacc.Bacc:
    global _NC_CACHE
    if _NC_CACHE is not None:
        return _NC_CACHE

    nc = bacc.Bacc(trn_type="TRN2", monotonic_sem_count=0)
    feat = nc.dram_tensor("feature", [R, C], mybir.dt.float32, kind="ExternalInput")
    # labpack[0] = labels, labpack[1] = arange(R)*C (constant row offsets).
    lab = nc.dram_tensor("labpack", [2, R], mybir.dt.int32, kind="ExternalInput")
    out = nc.dram_tensor("out", [1, 1], mybir.dt.float32, kind="ExternalOutput")

    H = R // 2  # picks per gather chunk

    with (
        nc.sbuf_tensor("lp", [P, 2, K], mybir.dt.int32) as lp,
        nc.sbuf_tensor("idx", [P, K], mybir.dt.int32) as idx,
        nc.sbuf_tensor("gat", [1, R], mybir.dt.float32) as gat,
        nc.sbuf_tensor("res2", [1, 2], mybir.dt.float32) as res2,
        nc.sbuf_tensor("res", [1, 1], mybir.dt.float32) as res,
        nc.semaphore() as S,
        nc.semaphore() as S2,
        nc.Block(no_gpsimd_drain=True) as block,
    ):
        # S timeline: 16 labpack DMA done; 32 gather chunk 0 done; 48 chunk 1
        # done; 49/50 chunk reduces, 51 final combine. The out DMA carries no
        # inc — the Scalar engine's block-exit Drain guarantees completion.
        # SP clears S once computation is done so re-executions of the loaded
        # NEFF start from zero (NRT only zeroes sems at load).

        @block.sync
        def _(sp):
            # Local row r = p*K + j lives at tile position [p, :, j].
            sp.dma_start(
                out=lp[:], in_=lab[:].rearrange("t (p k) -> p t k", p=P)
            ).then_inc(S, 16)
            sp.wait_ge(S, 51)
            sp.sem_clear(S)

        @block.gpsimd
        def _(g):
            g.wait_ge(S, 16)
            # Flat element index of feature[r, label[r]] = r*C + label[r].
            g.tensor_tensor(
                out=idx[:],
                in0=lp[:, 0, :],
                in1=lp[:, 1, :],
                op=mybir.AluOpType.add,
            )
            # Indirect gathers, chunked so the first chunk's reduce hides
            # under the second chunk's transfer. Walrus generates one
            # descriptor per entry of the dest AP's second-to-last dim, so a
            # [1, H, 1] dest on one partition yields H single-element
            # descriptors; the offset tile is consumed partition-fastest
            # (a permutation of our [p, j] order — irrelevant for the sum).
            # qPoolDynamic descriptors are ring-ordered across the 16 DMA
            # engines, so chunk sems complete in issue order.
            for c in range(2):
                g.indirect_dma_start(
                    out=gat[0:1, c * H : (c + 1) * H].rearrange(
                        "p (n one) -> p n one", one=1
                    ),
                    out_offset=None,
                    in_=feat[:],
                    # axis=1 -> coef = prod(shape[2:]) = 1: indices are flat
                    # element offsets into the contiguous [R, C] block.
                    in_offset=bass.IndirectOffsetOnAxis(
                        ap=idx[:, c * (K // 2) : (c + 1) * (K // 2)], axis=1
                    ),
                ).then_inc(S, 16)

        @block.vector
        def _(v):
            for c in range(2):
                v.wait_ge(S, 32 + 16 * c)
                v.tensor_reduce(
                    out=res2[0:1, c : c + 1],
                    in_=gat[0:1, c * H : (c + 1) * H],
                    axis=mybir.AxisListType.X,
                    op=mybir.AluOpType.add,
                ).then_inc(S, 1)
            v.tensor_reduce(
                out=res[:],
                in_=res2[:],
                axis=mybir.AxisListType.X,
                op=mybir.AluOpType.add,
            ).then_inc(S, 1)

        @block.scalar
        def _(a):
            a.wait_ge(S, 51)
            # Walrus requires a completion update on every DMA; S2 is a
            # write-only counter nobody waits on (completion is guaranteed
            # by the Scalar engine's block-exit Drain), so it needs no
            # end-of-run clear and the critical path skips the out-DMA
            # completion latency.
            a.dma_start(out=out[:], in_=res[:]).then_inc(S2, 16)

    nc.finalize()
    _NC_CACHE = nc
    return nc


def _run(feature: np.ndarray, label: np.ndarray, **spmd_kwargs):
    nc = _build_nc()
    feature = np.ascontiguousarray(feature, dtype=np.float32)
    lab32 = np.ascontiguousarray(np.asarray(label).astype(np.int32))
    assert feature.shape == (N, C), feature.shape
    assert lab32.shape == (N,), lab32.shape

    row_off = (np.arange(R, dtype=np.int32) * C).astype(np.int32)
    in_maps = [
        {
            "feature": feature[c * R : (c + 1) * R],
            "labpack": np.stack([lab32[c * R : (c + 1) * R], row_off]),
        }
        for c in range(N_CORES)
    ]
    res = bass_utils.run_bass_kernel_spmd(
        nc, in_maps, core_ids=list(range(N_CORES)), **spmd_kwargs
    )
    partials = np.array(
        [m["out"].reshape(()) for m in res.results], dtype=np.float32
    )
    total = np.float32(partials.sum(dtype=np.float32))
    loss = np.float32(2.0) - np.float32(2.0) * (total / np.float32(SCALE)) / np.float32(N)
    return np.asarray(loss, dtype=np.float32), res


def kernel(feature: np.ndarray, label: np.ndarray) -> np.ndarray:
    loss, _ = _run(feature, label)
    return loss


# revision 25
# speedup vs baseline: 1.3502x; 1.0221x over previous
"""Center-loss kernel for Trainium2 (Bass/Bacc, raw), 8-core data-parallel.

loss = 2 - 2 * (sum_i feature[i, label[i]] / 64) / 8192

Sharding: batch dim (8192 rows) split evenly across 8 NeuronCores.
Each core gathers its 1024 picked elements straight out of DRAM with
indirect DMAs (no full-matrix read), reduces them to one scalar partial
sum on-device, and the host combines the 8 partials.

Implementation notes (hard-won on this toolchain):
- Raw Bacc instead of TileContext: Tile's sem-init preamble + tail
  barrier butterfly cost ~16us on a ~4us kernel. Everything here runs
  on the GPSIMD (Pool) engine in-order with ONE DMA semaphore.
- The HW indirect DMA consumes ONE index per dest partition and fetches
  dest-free-size contiguous elements, so gathering 1024 scattered
  elements takes 8 DMAs of [128,1] (index tile column by column).
- labpack packs labels + constant row offsets so the index add is a
  single tensor_tensor with one sem wait (HW: 1 sync wait per inst).
"""

import sys

if "/opt/trn_rl_repo" not in sys.path:
    sys.path.insert(0, "/opt/trn_rl_repo")

import numpy as np

import concourse.bacc as bacc
import concourse.bass as bass
from concourse import mybir
from concourse import bass_utils

N = 8192          # batch rows
C = 10000         # num classes (feature columns)
N_CORES = 8
R = N // N_CORES  # rows per core
P = 128           # SBUF partitions
K = R // P        # picked elements per partition
SCALE = 64.0

_NC_CACHE = None


def _build_nc() -> bacc.Bacc:
    global _NC_CACHE
    if _NC_CACHE is not None:
        return _NC_CACHE

    nc = bacc.Bacc(trn_type="TRN2", monotonic_sem_count=0)
    feat = nc.dram_tensor("feature", [R, C], mybir.dt.float32, kind="ExternalInput")
    # labpack[0] = labels, labpack[1] = arange(R)*C (constant row offsets).
    lab = nc.dram_tensor("labpack", [2, R], mybir.dt.int32, kind="ExternalInput")
    out = nc.dram_tensor("out", [1, 1], mybir.dt.float32, kind="ExternalOutput")

    H = R // 2  # picks per gather chunk

    with (
        nc.sbuf_tensor("lp", [P, 2, K], mybir.dt.int32) as lp,
        nc.sbuf_tensor("idx", [P, K], mybir.dt.int32) as idx,
        nc.sbuf_tensor("gat", [1, R], mybir.dt.float32) as gat,
        nc.sbuf_tensor("res2", [1, 2], mybir.dt.float32) as res2,
        nc.sbuf_tensor("res", [1, 1], mybir.dt.float32) as res,
        nc.semaphore() as S,
        nc.semaphore() as SA,
        nc.semaphore() as SB,
        nc.semaphore() as S2,
        nc.Block(no_gpsimd_drain=True) as block,
    ):
        # S: 16 labpack DMA done, 17 all reduces done. SA/SB: gather chunk
        # 0/1 done (separate sems — qPoolDynamic chunk completions are NOT
        # ordered). The out DMA only incs S2, a write-only counter nobody
        # waits on (completion is guaranteed by the Scalar engine's
        # block-exit Drain), so the critical path skips its completion
        # latency and S2 needs no clear. SP clears the waited-on sems once
        # compute is done so re-executions of the loaded NEFF start from
        # zero (NRT only zeroes sems at load).

        @block.sync
        def _(sp):
            # Local row r = p*K + j lives at tile position [p, :, j].
            sp.dma_start(
                out=lp[:], in_=lab[:].rearrange("t (p k) -> p t k", p=P)
            ).then_inc(S, 16)
            sp.wait_ge(S, 17)
            sp.sem_clear(S)
            sp.sem_clear(SA)
            sp.sem_clear(SB)

        @block.gpsimd
        def _(g):
            g.wait_ge(S, 16)
            # Flat element index of feature[r, label[r]] = r*C + label[r].
            g.tensor_tensor(
                out=idx[:],
                in0=lp[:, 0, :],
                in1=lp[:, 1, :],
                op=mybir.AluOpType.add,
            )
            # Indirect gathers, chunked so the first chunk's reduce hides
            # under the second chunk's transfer. Walrus generates one
            # descriptor per entry of the dest AP's second-to-last dim, so a
            # [1, H, 1] dest on one partition yields H single-element
            # descriptors; the offset tile is consumed partition-fastest
            # (a permutation of our [p, j] order — irrelevant for the sum).
            for c, sem in ((0, SA), (1, SB)):
                g.indirect_dma_start(
                    out=gat[0:1, c * H : (c + 1) * H].rearrange(
                        "p (n one) -> p n one", one=1
                    ),
                    out_offset=None,
                    in_=feat[:],
                    # axis=1 -> coef = prod(shape[2:]) = 1: indices are flat
                    # element offsets into the contiguous [R, C] block.
                    in_offset=bass.IndirectOffsetOnAxis(
                        ap=idx[:, c * (K // 2) : (c + 1) * (K // 2)], axis=1
                    ),
                ).then_inc(sem, 16)

        @block.vector
        def _(v):
            for c, sem in ((0, SA), (1, SB)):
                v.wait_ge(sem, 16)
                v.tensor_reduce(
                    out=res2[0:1, c : c + 1],
                    in_=gat[0:1, c * H : (c + 1) * H],
                    axis=mybir.AxisListType.X,
                    op=mybir.AluOpType.add,
                )
            v.tensor_reduce(
                out=res[:],
                in_=res2[:],
                axis=mybir.AxisListType.X,
                op=mybir.AluOpType.add,
            ).then_inc(S, 1)

        @block.scalar
        def _(a):
            a.wait_ge(S, 17)
            # Walrus requires a completion update on every DMA; S2 is a
            # write-only counter nobody waits on (completion is guaranteed
            # by the Scalar engine's block-exit Drain), so it needs no
            # end-of-run clear and the critical path skips the out-DMA
            # completion latency.
            a.dma_start(out=out[:], in_=res[:]).then_inc(S2, 16)

    nc.finalize()
    _NC_CACHE = nc
    return nc


def _run(feature: np.ndarray, label: np.ndarray, **spmd_kwargs):
    nc = _build_nc()
    feature = np.ascontiguousarray(feature, dtype=np.float32)
    lab32 = np.ascontiguousarray(np.asarray(label).astype(np.int32))
    assert feature.shape == (N, C), feature.shape
    assert lab32.shape == (N,), lab32.shape

    row_off = (np.arange(R, dtype=np.int32) * C).astype(np.int32)
    in_maps = [
        {
            "feature": feature[c * R : (c + 1) * R],
            "labpack": np.stack([lab32[c * R : (c + 1) * R], row_off]),
        }
        for c in range(N_CORES)
    ]
    res = bass_utils.run_bass_kernel_spmd(
        nc, in_maps, core_ids=list(range(N_CORES)), **spmd_kwargs
    )
    partials = np.array(
        [m["out"].reshape(()) for m in res.results], dtype=np.float32
    )
    total = np.float32(partials.sum(dtype=np.float32))
    loss = np.float32(2.0) - np.float32(2.0) * (total / np.float32(SCALE)) / np.float32(N)
    return np.asarray(loss, dtype=np.float32), res


def kernel(feature: np.ndarray, label: np.ndarray) -> np.ndarray:
    loss, _ = _run(feature, label)
    return loss


# revision 26
# speedup vs baseline: 1.3567x; 1.0048x over previous
"""Center-loss kernel for Trainium2 (Bass/Bacc, raw), 8-core data-parallel.

loss = 2 - 2 * (sum_i feature[i, label[i]] / 64) / 8192

Sharding: batch dim (8192 rows) split evenly across 8 NeuronCores.
Each core gathers its 1024 picked elements straight out of DRAM with
indirect DMAs (no full-matrix read), reduces them to one scalar partial
sum on-device, and the host combines the 8 partials.

Implementation notes (hard-won on this toolchain):
- Raw Bacc instead of TileContext: Tile's sem-init preamble + tail
  barrier butterfly cost ~16us on a ~4us kernel. Everything here runs
  on the GPSIMD (Pool) engine in-order with ONE DMA semaphore.
- The HW indirect DMA consumes ONE index per dest partition and fetches
  dest-free-size contiguous elements, so gathering 1024 scattered
  elements takes 8 DMAs of [128,1] (index tile column by column).
- labpack packs labels + constant row offsets so the index add is a
  single tensor_tensor with one sem wait (HW: 1 sync wait per inst).
"""

import sys

if "/opt/trn_rl_repo" not in sys.path:
    sys.path.insert(0, "/opt/trn_rl_repo")

import numpy as np

import concourse.bacc as bacc
import concourse.bass as bass
from concourse import mybir
from concourse import bass_utils

N = 8192          # batch rows
C = 10000         # num classes (feature columns)
N_CORES = 8
R = N // N_CORES  # rows per core
P = 128           # SBUF partitions
K = R // P        # picked elements per partition
SCALE = 64.0

_NC_CACHE = None


def _build_nc() -> bacc.Bacc:
    global _NC_CACHE
    if _NC_CACHE is not None:
        return _NC_CACHE

    nc = bacc.Bacc(trn_type="TRN2", monotonic_sem_count=0)
    feat = nc.dram_tensor("feature", [R, C], mybir.dt.float32, kind="ExternalInput")
    # labpack[0] = labels, labpack[1] = arange(R)*C (constant row offsets).
    lab = nc.dram_tensor("labpack", [2, R], mybir.dt.int32, kind="ExternalInput")
    out = nc.dram_tensor("out", [1, 1], mybir.dt.float32, kind="ExternalOutput")

    H = R // 2  # picks per gather chunk

    with (
        nc.sbuf_tensor("lp", [P, 2, K], mybir.dt.int32) as lp,
        nc.sbuf_tensor("idx", [P, K], mybir.dt.int32) as idx,
        nc.sbuf_tensor("gat", [1, R], mybir.dt.float32) as gat,
        nc.sbuf_tensor("res2", [1, 2], mybir.dt.float32) as res2,
        nc.sbuf_tensor("res", [1, 1], mybir.dt.float32) as res,
        nc.semaphore() as S,
        nc.semaphore() as SA,
        nc.semaphore() as SB,
        nc.semaphore() as S2,
        nc.Block(no_gpsimd_drain=True) as block,
    ):
        # S: 16 labpack DMA done, 17 all reduces done. SA/SB: gather chunk
        # 0/1 done (separate sems — qPoolDynamic chunk completions are NOT
        # ordered). The out DMA only incs S2, a write-only counter nobody
        # waits on (completion is guaranteed by the Scalar engine's
        # block-exit Drain), so the critical path skips its completion
        # latency and S2 needs no clear. SP clears the waited-on sems once
        # compute is done so re-executions of the loaded NEFF start from
        # zero (NRT only zeroes sems at load).

        @block.sync
        def _(sp):
            # Local row r = p*K + j lives at tile position [p, :, j].
            sp.dma_start(
                out=lp[:], in_=lab[:].rearrange("t (p k) -> p t k", p=P)
            ).then_inc(S, 16)
            sp.wait_ge(S, 17)
            sp.sem_clear(S)
            sp.sem_clear(SA)
            sp.sem_clear(SB)

        @block.gpsimd
        def _(g):
            g.wait_ge(S, 16)
            # Flat element index of feature[r, label[r]] = r*C + label[r].
            g.tensor_tensor(
                out=idx[:],
                in0=lp[:, 0, :],
                in1=lp[:, 1, :],
                op=mybir.AluOpType.add,
            )
            # Indirect gathers, chunked so the first chunk's reduce hides
            # under the second chunk's transfer. Walrus generates one
            # descriptor per entry of the dest AP's second-to-last dim, so a
            # [1, H, 1] dest on one partition yields H single-element
            # descriptors; the offset tile is consumed partition-fastest
            # (a permutation of our [p, j] order — irrelevant for the sum).
            for c, sem in ((0, SA), (1, SB)):
                g.indirect_dma_start(
                    out=gat[0:1, c * H : (c + 1) * H].rearrange(
                        "p (n one) -> p n one", one=1
                    ),
                    out_offset=None,
                    in_=feat[:],
                    # axis=1 -> coef = prod(shape[2:]) = 1: indices are flat
                    # element offsets into the contiguous [R, C] block.
                    in_offset=bass.IndirectOffsetOnAxis(
                        ap=idx[:, c * (K // 2) : (c + 1) * (K // 2)], axis=1
                    ),
                ).then_inc(sem, 16)

        @block.vector
        def _(v):
            for c, sem in ((0, SA), (1, SB)):
                v.wait_ge(sem, 16)
                v.tensor_reduce(
                    out=res2[0:1, c : c + 1],
                    in_=gat[0:1, c * H : (c + 1) * H],
                    axis=mybir.AxisListType.X,
                    op=mybir.AluOpType.add,
                )
            # (An X-reduce of [1, 2] silently returns element 0 on this
            # toolchain — combine with a plain add instead.)
            v.tensor_tensor(
                out=res[:],
                in0=res2[0:1, 0:1],
                in1=res2[0:1, 1:2],
                op=mybir.AluOpType.add,
            ).then_inc(S, 1)

        @block.scalar
        def _(a):
            a.wait_ge(S, 17)
            # Walrus requires a completion update on every DMA; S2 is a
            # write-only counter nobody waits on (completion is guaranteed
            # by the Scalar engine's block-exit Drain), so it needs no
            # end-of-run clear and the critical path skips the out-DMA
            # completion latency.
            a.dma_start(out=out[:], in_=res[:]).then_inc(S2, 16)

    nc.finalize()
    _NC_CACHE = nc
    return nc


def _run(feature: np.ndarray, label: np.ndarray, **spmd_kwargs):
    nc = _build_nc()
    feature = np.ascontiguousarray(feature, dtype=np.float32)
    lab32 = np.ascontiguousarray(np.asarray(label).astype(np.int32))
    assert feature.shape == (N, C), feature.shape
    assert lab32.shape == (N,), lab32.shape

    row_off = (np.arange(R, dtype=np.int32) * C).astype(np.int32)
    in_maps = [
        {
            "feature": feature[c * R : (c + 1) * R],
            "labpack": np.stack([lab32[c * R : (c + 1) * R], row_off]),
        }
        for c in range(N_CORES)
    ]
    res = bass_utils.run_bass_kernel_spmd(
        nc, in_maps, core_ids=list(range(N_CORES)), **spmd_kwargs
    )
    partials = np.array(
        [m["out"].reshape(()) for m in res.results], dtype=np.float32
    )
    total = np.float32(partials.sum(dtype=np.float32))
    loss = np.float32(2.0) - np.float32(2.0) * (total / np.float32(SCALE)) / np.float32(N)
    return np.asarray(loss, dtype=np.float32), res


def kernel(feature: np.ndarray, label: np.ndarray) -> np.ndarray:
    loss, _ = _run(feature, label)
    return loss
